# revision 24
# baseline (speedup 1.0000x reference)
"""Trainium2 Bass kernel for nn_GatedShortConvBlock.

Full inputs -> full output. Data-parallel over batch: batch b -> core b.
Per-core dataflow (T=4096 tokens, D=1024, H=2048, K=4), TBB=1024 t-blocks:
  LN stats on ACT (accum_out sums over [t,d] tiles) -> tiny row math ->
  per-token rstd / mu*rstd rows broadcast to [128,TBB] via GpSimd
  partition_broadcast -> normalize+quantize xT to fp8e4 on DVE ->
  in-proj as fp8e4 DoubleRow matmuls (contraction pairs of 128-chunks,
  weights SBUF-resident, pre-scaled x1024 on host) -> gates via ACT
  sigmoid (descale folded into activation scale) -> depthwise causal
  conv on DVE as tensor_scalar(4x)/tensor_tensor(2x) bf16 chain ->
  out-proj fp8-DoubleRow (OUT_FP8) or bf16 (not) -> drain via one
  scalar_tensor_tensor (descale + residual(x+b_out)) -> DMA from PSUM.
"""

import sys
import types

import numpy as np

B, T, D, H = 8, 4096, 1024, 2048
KCONV = 4
N_CORES = 8
TBB = 1024          # tokens per t-block
NTB = T // TBB      # 4
NSUB = TBB // 128   # 8 (128-token subtiles per block)
KC = H // 128       # 16 channel blocks
DC = D // 128       # 8 d-chunks
EPS = 1e-5
S_IN = 1024.0       # host pre-scale on w_in (power of 2)
S_OUT = 1024.0      # host pre-scale on w_out
OUT_FP8 = False     # fp8-DoubleRow out-proj; False -> bf16 out-proj

_CACHE = {}


def _install_ntff_hook():
    if "antenv.axon_hooks" in sys.modules:
        return
    try:
        import trn_agent_boot.trn_boot as tb

        hook = tb._ntff_profile_via_ctypes("/opt/axon/libaxon_pjrt.so")
        mod = types.ModuleType("antenv.axon_hooks")
        mod.get_axon_ntff_profile_hook = lambda: hook
        mod.set_axon_ntff_profile_hook = lambda h: None
        sys.modules["antenv.axon_hooks"] = mod
    except Exception:
        pass


def _build_nc(num_devices=N_CORES):
    import concourse.bass as bass  # noqa: F401
    import concourse.tile as tile
    from concourse import bacc, mybir

    f32 = mybir.dt.float32
    bf16 = mybir.dt.bfloat16
    f8 = mybir.dt.float8e4
    u8 = mybir.dt.uint8
    u32 = mybir.dt.uint32
    AF = mybir.ActivationFunctionType
    OP = mybir.AluOpType
    DR = mybir.MatmulPerfMode.DoubleRow

    nc = bacc.Bacc("TRN2", target_bir_lowering=False, debug=False,
                   num_devices=num_devices)
    x_d = nc.dram_tensor("x", [T, D], f32, kind="ExternalInput").ap()
    xpb_d = nc.dram_tensor("xpb", [T, D], f32, kind="ExternalInput").ap()
    xt_d = nc.dram_tensor("xt_proc", [NTB, DC, 128, TBB], bf16,
                          kind="ExternalInput").ap()
    win8_d = nc.dram_tensor("w_in8", [128, KC, 3, DC // 2, 2, 128], u8,
                            kind="ExternalInput").ap()
    if OUT_FP8:
        wout_d = nc.dram_tensor("w_out8", [128, KC // 2, 2, D], u8,
                                kind="ExternalInput").ap()
    else:
        wout_d = nc.dram_tensor("w_outb", [128, KC, D], bf16,
                                kind="ExternalInput").ap()
    d_d = nc.dram_tensor("d_proc", [128, KC, 3], f32,
                         kind="ExternalInput").ap()
    cw_d = nc.dram_tensor("cw_proc", [128, KC, KCONV], f32,
                          kind="ExternalInput").ap()
    cb_d = nc.dram_tensor("cb_proc", [128, KC], f32,
                          kind="ExternalInput").ap()
    iden_d = nc.dram_tensor("iden", [128, 128], f32,
                            kind="ExternalInput").ap()
    out_d = nc.dram_tensor("out", [T, D], f32, kind="ExternalOutput").ap()

    from contextlib import ExitStack

    with tile.TileContext(nc) as tc, ExitStack() as ctx:
        consts = ctx.enter_context(tc.tile_pool(name="consts", bufs=1))
        xtp = ctx.enter_context(tc.tile_pool(name="xtp", bufs=2))
        xsp = ctx.enter_context(tc.tile_pool(name="xsp", bufs=2))
        x8p = ctx.enter_context(tc.tile_pool(name="x8p", bufs=2))
        sumsp = ctx.enter_context(tc.tile_pool(name="sumsp", bufs=2))
        rowp = ctx.enter_context(tc.tile_pool(name="rowp", bufs=1))
        row2p = ctx.enter_context(tc.tile_pool(name="row2p", bufs=3))
        bcp = ctx.enter_context(tc.tile_pool(name="bcp", bufs=2))
        scp = ctx.enter_context(tc.tile_pool(name="scp", bufs=2))
        sgp = ctx.enter_context(tc.tile_pool(name="sgp", bufs=2))
        cvp = ctx.enter_context(tc.tile_pool(name="cvp", bufs=5))
        otp = ctx.enter_context(tc.tile_pool(name="otp", bufs=2))
        psB = ctx.enter_context(tc.tile_pool(name="psB", bufs=3, space="PSUM"))
        psC = ctx.enter_context(tc.tile_pool(name="psC", bufs=2, space="PSUM"))

        # resident weights / constants
        win8_sb = consts.tile([128, KC, 3, DC // 2, 2, 128], u8)
        nc.sync.dma_start(out=win8_sb, in_=win8_d)
        if OUT_FP8:
            wout_sb = consts.tile([128, KC // 2, 2, D], u8)
        else:
            wout_sb = consts.tile([128, KC, D], bf16)
        nc.sync.dma_start(out=wout_sb, in_=wout_d)
        d_sb = consts.tile([128, KC, 3], f32)
        nc.sync.dma_start(out=d_sb, in_=d_d)
        cw_sb = consts.tile([128, KC, KCONV], f32)
        nc.sync.dma_start(out=cw_sb, in_=cw_d)
        cb_sb = consts.tile([128, KC], f32)
        nc.sync.dma_start(out=cb_sb, in_=cb_d)
        iden_sb = consts.tile([128, 128], f32)
        nc.sync.dma_start(out=iden_sb, in_=iden_d)
        hidb = consts.tile([128, KC, TBB + 3], bf16)
        nc.vector.memset(hidb[:, :, 0:3], 0.0)
        eps8 = consts.tile([NSUB, 1], f32)
        nc.vector.memset(eps8, EPS)

        hid2 = consts.tile([128, KC, TBB], u8 if OUT_FP8 else bf16)

        inv_s = 1.0 / S_IN

        def emit_stats_s(tb, s):
            t0 = tb * TBB
            xs = xsp.tile([128, D], f32, name="xs")
            nc.sync.dma_start(
                out=xs, in_=x_d[t0 + s * 128:t0 + (s + 1) * 128, :])
            scr = scp.tile([128, TBB], bf16, name="sc")
            nc.scalar.activation(out=scr[:, 0:D], in_=xs, func=AF.Copy,
                                 accum_out=sums_t[tb][:, 0, s:s + 1])
            nc.scalar.activation(out=scr[:, 0:D], in_=xs, func=AF.Square,
                                 accum_out=sums_t[tb][:, 1, s:s + 1])

        def emit_rows(tb):
            sums = sums_t[tb]
            pt = psC.tile([128, 512], f32, name="pc")
            nc.tensor.transpose(pt[0:NSUB, 0:128], sums[:, 0, :], iden_sb)
            nc.tensor.transpose(pt[0:NSUB, 128:256], sums[:, 1, :], iden_sb)
            rows = rowp.tile([NSUB, 256], bf16, name="rows")
            nc.scalar.copy(out=rows, in_=pt[0:NSUB, 0:256])
            mu = row2p.tile([NSUB, 128], f32, name="r2")
            nc.vector.tensor_scalar_mul(mu, rows[:, 0:128], 1.0 / D)
            ex2 = row2p.tile([NSUB, 128], f32, name="r2")
            nc.vector.tensor_scalar_mul(ex2, rows[:, 128:256], 1.0 / D)
            var = row2p.tile([NSUB, 128], f32, name="r2")
            nc.vector.scalar_tensor_tensor(
                out=var, in0=mu, scalar=-1.0, in1=mu,
                op0=OP.mult, op1=OP.mult)
            nc.vector.tensor_add(out=var, in0=var, in1=ex2)
            sd = row2p.tile([NSUB, 128], f32, name="r2")
            nc.scalar.activation(out=sd, in_=var, func=AF.Sqrt, bias=eps8,
                                 scale=1.0)
            rstd = row2p.tile([NSUB, 128], bf16, name="r2h")
            with nc.allow_low_precision(reason="rows feed fp8 quant"):
                nc.vector.reciprocal(out=rstd, in_=sd)
            mur = row2p.tile([NSUB, 128], bf16, name="r2h")
            nc.vector.tensor_mul(out=mur, in0=mu, in1=rstd)
            rrow = rowp.tile([1, TBB], bf16, name="rrow")
            nc.sync.dma_start(out=rrow, in_=rstd)
            mrow = rowp.tile([1, TBB], bf16, name="mrow")
            nc.sync.dma_start(out=mrow, in_=mur)
            r_b = bcp.tile([128, TBB], bf16, name="bc")
            mr_b = bcp.tile([128, TBB], bf16, name="bc")
            nc.gpsimd.partition_broadcast(r_b, rrow)
            nc.gpsimd.partition_broadcast(mr_b, mrow)
            bcast_t[tb] = (r_b, mr_b)

        def emit_xt_dma(tb, j):
            xT = xtp.tile([128, TBB], bf16, name="xT")
            nc.gpsimd.dma_start(out=xT, in_=xt_d[tb, j])
            xT_t[(tb, j)] = xT

        def emit_norm_j(tb, j):
            r_b, mr_b = bcast_t[tb]
            xT = xT_t.pop((tb, j))
            sc = scp.tile([128, TBB], bf16, name="sc")
            nc.vector.tensor_mul(out=sc, in0=xT, in1=r_b)
            nc.vector.tensor_sub(out=x8_t[tb][:, j, :].bitcast(f8),
                                 in0=sc, in1=mr_b)

        def emit_B_k(tb, k):
            x8 = x8_t[tb]
            ps = {}
            for t3 in (0, 2, 1):
                p = psB.tile([128, TBB], f32, name="mm")
                ps[t3] = p
                for j0 in range(DC // 2):
                    lw = win8_sb[:, k, t3, j0, :, :].bitcast(f8)
                    for n in range(2):
                        nc.tensor.matmul(
                            p[:, n * 512:(n + 1) * 512],
                            lw,
                            x8[:, 2 * j0:2 * j0 + 2,
                               n * 512:(n + 1) * 512].bitcast(f8),
                            start=(j0 == 0), stop=(j0 == DC // 2 - 1),
                            perf_mode=DR)
                if t3 == 0:
                    sigb = sgp.tile([128, TBB], bf16, name="sg")
                    nc.scalar.activation(
                        out=sigb, in_=ps[0], func=AF.Sigmoid,
                        bias=d_sb[:, k, 0:1], scale=inv_s)
                elif t3 == 2:
                    hp = cvp.tile([128, TBB], bf16, name="cv")
                    nc.scalar.activation(
                        out=hp, in_=ps[2], func=AF.Identity,
                        bias=d_sb[:, k, 2:3], scale=1.0)
                else:
                    sigc = sgp.tile([128, TBB], bf16, name="sg")
                    nc.scalar.activation(
                        out=sigc, in_=ps[1], func=AF.Sigmoid,
                        bias=d_sb[:, k, 1:2], scale=inv_s)
            nc.vector.tensor_mul(out=hidb[:, k, 3:3 + TBB],
                                 in0=hp, in1=sigb)
            m0 = cvp.tile([128, TBB], bf16, name="cv")
            nc.vector.tensor_scalar_mul(
                m0, hidb[:, k, 0:TBB], cw_sb[:, k, 0:1])
            m1 = cvp.tile([128, TBB], bf16, name="cv")
            nc.vector.tensor_scalar(
                out=m1, in0=hidb[:, k, 1:1 + TBB],
                scalar1=cw_sb[:, k, 1:2], scalar2=cb_sb[:, k:k + 1],
                op0=OP.mult, op1=OP.add)
            t0_ = cvp.tile([128, TBB], bf16, name="cv")
            nc.vector.tensor_add(out=t0_, in0=m0, in1=m1)
            m2 = cvp.tile([128, TBB], bf16, name="cv")
            nc.vector.tensor_scalar_mul(
                m2, hidb[:, k, 2:2 + TBB], cw_sb[:, k, 2:3])
            m3 = cvp.tile([128, TBB], bf16, name="cv")
            nc.vector.tensor_scalar_mul(
                m3, hidb[:, k, 3:3 + TBB], cw_sb[:, k, 3:4])
            t1_ = cvp.tile([128, TBB], bf16, name="cv")
            nc.vector.tensor_add(out=t1_, in0=m2, in1=m3)
            t2_ = cvp.tile([128, TBB], bf16, name="cv")
            nc.vector.tensor_add(out=t2_, in0=t0_, in1=t1_)
            h2out = hid2[:, k, :]
            if OUT_FP8:
                h2out = h2out.bitcast(f8)
            nc.vector.tensor_mul(out=h2out, in0=t2_, in1=sigc)

        def emit_C_s(tb, s):
            t0 = tb * TBB
            pr = [psC.tile([128, 512], f32, name="pc") for _ in range(2)]
            if OUT_FP8:
                for j0 in range(KC // 2):
                    lh = hid2[:, 2 * j0:2 * j0 + 2,
                              s * 128:(s + 1) * 128].bitcast(f8)
                    for dh in range(2):
                        nc.tensor.matmul(
                            pr[dh], lh,
                            wout_sb[:, j0, :,
                                    dh * 512:(dh + 1) * 512].bitcast(f8),
                            start=(j0 == 0), stop=(j0 == KC // 2 - 1),
                            perf_mode=DR)
            else:
                for j in range(KC):
                    lh = hid2[:, j, s * 128:(s + 1) * 128]
                    for dh in range(2):
                        nc.tensor.matmul(
                            pr[dh], lh,
                            wout_sb[:, j, dh * 512:(dh + 1) * 512],
                            start=(j == 0), stop=(j == KC - 1))
            xr = xsp.tile([128, D], f32, name="xs")
            nc.sync.dma_start(
                out=xr, in_=xpb_d[t0 + s * 128:t0 + (s + 1) * 128, :])
            for dh in range(2):
                ot = otp.tile([128, 512], f32, name="ot")
                nc.vector.scalar_tensor_tensor(
                    out=ot, in0=pr[dh],
                    scalar=(1.0 / S_OUT) if OUT_FP8 else 1.0,
                    in1=xr[:, dh * 512:(dh + 1) * 512],
                    op0=OP.mult, op1=OP.add)
                nc.sync.dma_start(
                    out=out_d[t0 + s * 128:t0 + (s + 1) * 128,
                              dh * 512:(dh + 1) * 512],
                    in_=ot)

        sums_t, bcast_t, xT_t, x8_t = {}, {}, {}, {}

        # prologue: full phase A for tb=0
        pw = psC.tile([128, 512], f32, name="pc")
        sums_t[0] = sumsp.tile([128, 2, NSUB], f32, name="sums")
        for s in range(NSUB):
            emit_stats_s(0, s)
        for i in range(90):
            nc.tensor.matmul(pw[:, 0:64], iden_sb, iden_sb[:, 0:64],
                             start=True, stop=True)
        emit_rows(0)
        for i in range(60):
            nc.tensor.matmul(pw[:, 0:64], iden_sb, iden_sb[:, 0:64],
                             start=True, stop=True)
        x8_t[0] = x8p.tile([128, DC, TBB], u8, name="x8")
        for j in range(DC):
            emit_xt_dma(0, j)
            emit_norm_j(0, j)

        for tb in range(NTB):
            nxt = tb + 1 if tb + 1 < NTB else None
            if nxt is not None:
                sums_t[nxt] = sumsp.tile([128, 2, NSUB], f32, name="sums")
            for k in range(KC):
                emit_B_k(tb, k)
                if nxt is not None:
                    if k < NSUB:
                        emit_stats_s(nxt, k)
                    elif k == NSUB:
                        emit_rows(nxt)
                        x8_t[nxt] = x8p.tile([128, DC, TBB], u8, name="x8")
                    elif k >= 9:
                        if k == 9:
                            emit_xt_dma(nxt, 0)
                        emit_xt_dma(nxt, k - 9 + 1)
            if nxt is not None:
                emit_xt_dma(nxt, DC - 1)
            for s in range(NSUB):
                emit_C_s(tb, s)
                if nxt is not None and s < 4:
                    emit_norm_j(nxt, 2 * s)
                    emit_norm_j(nxt, 2 * s + 1)
            x8_t.pop(tb)
            # conv halo: copy last 3 hidden tokens to block front
            nc.gpsimd.tensor_copy(out=hidb[:, :, 0:3],
                                  in_=hidb[:, :, TBB:TBB + 3])
    nc.compile()
    return nc


def _q8(a, scale):
    import ml_dtypes

    q = np.clip(a * scale, -240.0, 240.0).astype(ml_dtypes.float8_e4m3)
    return q.view(np.uint8)


def _prep_inputs(x, ln_g, ln_b, w_in, b_in, conv_w, conv_b, w_out, b_out):
    import ml_dtypes

    x = np.asarray(x, np.float32)
    ln_g = np.asarray(ln_g, np.float32)
    ln_b = np.asarray(ln_b, np.float32)
    w_in = np.asarray(w_in, np.float32)
    b_in = np.asarray(b_in, np.float32)
    conv_w = np.asarray(conv_w, np.float32)
    conv_b = np.asarray(conv_b, np.float32)
    w_out = np.asarray(w_out, np.float32)
    b_out = np.asarray(b_out, np.float32)

    wg = w_in * ln_g[None, :]                       # [3H, D]
    dv = b_in + w_in @ ln_b                         # [3H]
    # w_in8[p, k, t3, j0, i, m] = q8(wg[t3*2048+k*128+m, (2j0+i)*128+p]*S_IN)
    wblk = wg.reshape(3, KC, 128, DC // 2, 2, 128)  # [t3, k, m, j0, i, p]
    w_in8 = _q8(np.ascontiguousarray(wblk.transpose(5, 1, 0, 3, 4, 2)), S_IN)
    d_proc = np.ascontiguousarray(dv.reshape(3, KC, 128).transpose(2, 1, 0))
    d_proc[:, :, 2] *= S_IN
    cw_proc = np.ascontiguousarray(
        conv_w.reshape(KC, 128, KCONV).transpose(1, 0, 2)) / S_IN
    cb_proc = np.ascontiguousarray(conv_b.reshape(KC, 128).T)
    if OUT_FP8:
        # w_out8[p, j0, i, d] = q8(w_out[d, (2j0+i)*128+p] * S_OUT)
        wo = w_out.T.reshape(KC // 2, 2, 128, D)    # [j0, i, p, d]
        w_out_proc = _q8(np.ascontiguousarray(wo.transpose(2, 0, 1, 3)), S_OUT)
        wout_key = "w_out8"
    else:
        wo = w_out.T.reshape(KC, 128, D).transpose(1, 0, 2)
        w_out_proc = np.ascontiguousarray(wo).astype(ml_dtypes.bfloat16)
        wout_key = "w_outb"
    iden = np.eye(128, dtype=np.float32)

    shared = {
        "w_in8": w_in8, "d_proc": d_proc, "cw_proc": cw_proc,
        "cb_proc": cb_proc, wout_key: w_out_proc, "iden": iden,
    }
    maps = []
    for b in range(B):
        xb = np.ascontiguousarray(x[b])
        # xt_proc[tb, j, p, t] = x[b, TBB*tb + t, 128*j + p]
        xt_proc = np.ascontiguousarray(
            xb.T.reshape(DC, 128, NTB, TBB).transpose(2, 0, 1, 3)).astype(
                ml_dtypes.bfloat16)
        xpb = xb + b_out[None, :]
        maps.append(dict(shared, x=xb, xpb=xpb, xt_proc=xt_proc))
    return maps


def run(inputs, trace=False):
    _install_ntff_hook()
    import concourse.bass_utils as bu

    bu.upload_artifacts = lambda d: "local://" + d
    if "nc" not in _CACHE:
        _CACHE["nc"] = _build_nc()
    nc = _CACHE["nc"]
    in_maps = _prep_inputs(**inputs)
    res = bu.run_bass_kernel_spmd(nc, in_maps, list(range(N_CORES)),
                                  trace=trace)
    out = np.stack([res.results[b]["out"] for b in range(B)])
    return out.astype(np.float32), res


def kernel(**inputs):
    out, _ = run(inputs, trace=False)
    return out


# revision 25
# speedup vs baseline: 1.0505x; 1.0505x over previous
"""Trainium2 Bass kernel for nn_GatedShortConvBlock.

Full inputs -> full output. Data-parallel over batch: batch b -> core b.
Per-core dataflow (T=4096 tokens, D=1024, H=2048, K=4), TBB=1024 t-blocks:
  LN stats on ACT (accum_out sums over [t,d] tiles) -> tiny row math ->
  per-token rstd / mu*rstd rows broadcast to [128,TBB] via GpSimd
  partition_broadcast -> normalize+quantize xT to fp8e4 on DVE ->
  in-proj as fp8e4 DoubleRow matmuls (contraction pairs of 128-chunks,
  weights SBUF-resident, pre-scaled x1024 on host) -> gates via ACT
  sigmoid (descale folded into activation scale) -> depthwise causal
  conv on DVE as tensor_scalar(4x)/tensor_tensor(2x) bf16 chain ->
  out-proj fp8-DoubleRow (OUT_FP8) or bf16 (not) -> drain via one
  scalar_tensor_tensor (descale + residual(x+b_out)) -> DMA from PSUM.
"""

import sys
import types

import numpy as np

B, T, D, H = 8, 4096, 1024, 2048
KCONV = 4
N_CORES = 8
TBB = 1024          # tokens per t-block
NTB = T // TBB      # 4
NSUB = TBB // 128   # 8 (128-token subtiles per block)
KC = H // 128       # 16 channel blocks
DC = D // 128       # 8 d-chunks
EPS = 1e-5
S_IN = 1024.0       # host pre-scale on w_in (power of 2)
S_OUT = 1024.0      # host pre-scale on w_out
OUT_FP8 = False     # fp8-DoubleRow out-proj; False -> bf16 out-proj

_CACHE = {}


def _install_ntff_hook():
    if "antenv.axon_hooks" in sys.modules:
        return
    try:
        import trn_agent_boot.trn_boot as tb

        hook = tb._ntff_profile_via_ctypes("/opt/axon/libaxon_pjrt.so")
        mod = types.ModuleType("antenv.axon_hooks")
        mod.get_axon_ntff_profile_hook = lambda: hook
        mod.set_axon_ntff_profile_hook = lambda h: None
        sys.modules["antenv.axon_hooks"] = mod
    except Exception:
        pass


def _build_nc(num_devices=N_CORES):
    import concourse.bass as bass  # noqa: F401
    import concourse.tile as tile
    from concourse import bacc, mybir

    f32 = mybir.dt.float32
    bf16 = mybir.dt.bfloat16
    f8 = mybir.dt.float8e4
    u8 = mybir.dt.uint8
    u32 = mybir.dt.uint32
    AF = mybir.ActivationFunctionType
    OP = mybir.AluOpType
    DR = mybir.MatmulPerfMode.DoubleRow

    nc = bacc.Bacc("TRN2", target_bir_lowering=False, debug=False,
                   num_devices=num_devices)
    x_d = nc.dram_tensor("x", [T, D], f32, kind="ExternalInput").ap()
    xpb_d = nc.dram_tensor("xpb", [T, D], f32, kind="ExternalInput").ap()
    xt_d = nc.dram_tensor("xt_proc", [NTB, DC, 128, TBB], bf16,
                          kind="ExternalInput").ap()
    win8_d = nc.dram_tensor("w_in8", [128, KC, 3, DC // 2, 2, 128], u8,
                            kind="ExternalInput").ap()
    if OUT_FP8:
        wout_d = nc.dram_tensor("w_out8", [128, KC // 2, 2, D], u8,
                                kind="ExternalInput").ap()
    else:
        wout_d = nc.dram_tensor("w_outb", [128, KC, D], bf16,
                                kind="ExternalInput").ap()
    d_d = nc.dram_tensor("d_proc", [128, KC, 3], f32,
                         kind="ExternalInput").ap()
    cw_d = nc.dram_tensor("cw_proc", [128, KC, KCONV], f32,
                          kind="ExternalInput").ap()
    cb_d = nc.dram_tensor("cb_proc", [128, KC], f32,
                          kind="ExternalInput").ap()
    iden_d = nc.dram_tensor("iden", [128, 128], f32,
                            kind="ExternalInput").ap()
    out_d = nc.dram_tensor("out", [T, D], f32, kind="ExternalOutput").ap()

    from contextlib import ExitStack

    with tile.TileContext(nc) as tc, ExitStack() as ctx:
        consts = ctx.enter_context(tc.tile_pool(name="consts", bufs=1))
        xtp = ctx.enter_context(tc.tile_pool(name="xtp", bufs=2))
        xsp = ctx.enter_context(tc.tile_pool(name="xsp", bufs=2))
        x8p = ctx.enter_context(tc.tile_pool(name="x8p", bufs=2))
        sumsp = ctx.enter_context(tc.tile_pool(name="sumsp", bufs=2))
        rowp = ctx.enter_context(tc.tile_pool(name="rowp", bufs=1))
        row2p = ctx.enter_context(tc.tile_pool(name="row2p", bufs=3))
        bcp = ctx.enter_context(tc.tile_pool(name="bcp", bufs=2))
        scp = ctx.enter_context(tc.tile_pool(name="scp", bufs=2))
        sgp = ctx.enter_context(tc.tile_pool(name="sgp", bufs=2))
        cvp = ctx.enter_context(tc.tile_pool(name="cvp", bufs=5))
        otp = ctx.enter_context(tc.tile_pool(name="otp", bufs=2))
        psB = ctx.enter_context(tc.tile_pool(name="psB", bufs=3, space="PSUM"))
        psC = ctx.enter_context(tc.tile_pool(name="psC", bufs=2, space="PSUM"))

        # resident weights / constants
        win8_sb = consts.tile([128, KC, 3, DC // 2, 2, 128], u8)
        nc.sync.dma_start(out=win8_sb, in_=win8_d)
        if OUT_FP8:
            wout_sb = consts.tile([128, KC // 2, 2, D], u8)
        else:
            wout_sb = consts.tile([128, KC, D], bf16)
        nc.sync.dma_start(out=wout_sb, in_=wout_d)
        d_sb = consts.tile([128, KC, 3], f32)
        nc.sync.dma_start(out=d_sb, in_=d_d)
        cw_sb = consts.tile([128, KC, KCONV], f32)
        nc.sync.dma_start(out=cw_sb, in_=cw_d)
        cb_sb = consts.tile([128, KC], f32)
        nc.sync.dma_start(out=cb_sb, in_=cb_d)
        iden_sb = consts.tile([128, 128], f32)
        nc.sync.dma_start(out=iden_sb, in_=iden_d)
        hidb = consts.tile([128, KC, TBB + 3], bf16)
        nc.vector.memset(hidb[:, :, 0:3], 0.0)
        eps8 = consts.tile([NSUB, 1], f32)
        nc.vector.memset(eps8, EPS)

        hid2 = consts.tile([128, KC, TBB], u8 if OUT_FP8 else bf16)

        inv_s = 1.0 / S_IN

        def emit_stats_s(tb, s):
            t0 = tb * TBB
            xs = xsp.tile([128, D], f32, name="xs")
            nc.sync.dma_start(
                out=xs, in_=x_d[t0 + s * 128:t0 + (s + 1) * 128, :])
            scr = scp.tile([128, TBB], bf16, name="sc")
            nc.scalar.activation(out=scr[:, 0:D], in_=xs, func=AF.Copy,
                                 accum_out=sums_t[tb][:, 0, s:s + 1])
            nc.scalar.activation(out=scr[:, 0:D], in_=xs, func=AF.Square,
                                 accum_out=sums_t[tb][:, 1, s:s + 1])

        def emit_rows(tb):
            sums = sums_t[tb]
            pt = psC.tile([128, 512], f32, name="pc")
            nc.tensor.transpose(pt[0:NSUB, 0:128], sums[:, 0, :], iden_sb)
            nc.tensor.transpose(pt[0:NSUB, 128:256], sums[:, 1, :], iden_sb)
            rows = rowp.tile([NSUB, 256], bf16, name="rows")
            nc.scalar.copy(out=rows, in_=pt[0:NSUB, 0:256])
            mu = row2p.tile([NSUB, 128], f32, name="r2")
            nc.vector.tensor_scalar_mul(mu, rows[:, 0:128], 1.0 / D)
            ex2 = row2p.tile([NSUB, 128], f32, name="r2")
            nc.vector.tensor_scalar_mul(ex2, rows[:, 128:256], 1.0 / D)
            var = row2p.tile([NSUB, 128], f32, name="r2")
            nc.vector.scalar_tensor_tensor(
                out=var, in0=mu, scalar=-1.0, in1=mu,
                op0=OP.mult, op1=OP.mult)
            nc.vector.tensor_add(out=var, in0=var, in1=ex2)
            sd = row2p.tile([NSUB, 128], f32, name="r2")
            nc.scalar.activation(out=sd, in_=var, func=AF.Sqrt, bias=eps8,
                                 scale=1.0)
            rstd = row2p.tile([NSUB, 128], bf16, name="r2h")
            with nc.allow_low_precision(reason="rows feed fp8 quant"):
                nc.vector.reciprocal(out=rstd, in_=sd)
            mur = row2p.tile([NSUB, 128], bf16, name="r2h")
            nc.vector.tensor_mul(out=mur, in0=mu, in1=rstd)
            rrow = rowp.tile([1, TBB], bf16, name="rrow")
            nc.sync.dma_start(out=rrow, in_=rstd)
            mrow = rowp.tile([1, TBB], bf16, name="mrow")
            nc.sync.dma_start(out=mrow, in_=mur)
            r_b = bcp.tile([128, TBB], bf16, name="bc")
            mr_b = bcp.tile([128, TBB], bf16, name="bc")
            nc.gpsimd.partition_broadcast(r_b, rrow)
            nc.gpsimd.partition_broadcast(mr_b, mrow)
            bcast_t[tb] = (r_b, mr_b)

        def emit_xt_dma(tb, j):
            xT = xtp.tile([128, TBB], bf16, name="xT")
            nc.gpsimd.dma_start(out=xT, in_=xt_d[tb, j])
            xT_t[(tb, j)] = xT

        def emit_norm_j(tb, j):
            r_b, mr_b = bcast_t[tb]
            xT = xT_t.pop((tb, j))
            sc = scp.tile([128, TBB], bf16, name="sc")
            nc.vector.tensor_mul(out=sc, in0=xT, in1=r_b)
            nc.vector.tensor_sub(out=x8_t[tb][:, j, :].bitcast(f8),
                                 in0=sc, in1=mr_b)

        def emit_B_k(tb, k):
            x8 = x8_t[tb]
            ps = {}
            for t3 in (0, 2, 1):
                p = psB.tile([128, TBB], f32, name="mm")
                ps[t3] = p
                for j0 in range(DC // 2):
                    lw = win8_sb[:, k, t3, j0, :, :].bitcast(f8)
                    for n in range(2):
                        nc.tensor.matmul(
                            p[:, n * 512:(n + 1) * 512],
                            lw,
                            x8[:, 2 * j0:2 * j0 + 2,
                               n * 512:(n + 1) * 512].bitcast(f8),
                            start=(j0 == 0), stop=(j0 == DC // 2 - 1),
                            perf_mode=DR)
                if t3 == 0:
                    sigb = sgp.tile([128, TBB], bf16, name="sg")
                    nc.scalar.activation(
                        out=sigb, in_=ps[0], func=AF.Sigmoid,
                        bias=d_sb[:, k, 0:1], scale=inv_s)
                elif t3 == 2:
                    hp = cvp.tile([128, TBB], bf16, name="cv")
                    nc.scalar.activation(
                        out=hp, in_=ps[2], func=AF.Identity,
                        bias=d_sb[:, k, 2:3], scale=1.0)
                else:
                    sigc = sgp.tile([128, TBB], bf16, name="sg")
                    nc.scalar.activation(
                        out=sigc, in_=ps[1], func=AF.Sigmoid,
                        bias=d_sb[:, k, 1:2], scale=inv_s)
            nc.vector.tensor_mul(out=hidb[:, k, 3:3 + TBB],
                                 in0=hp, in1=sigb)
            m0 = cvp.tile([128, TBB], bf16, name="cv")
            nc.vector.tensor_scalar_mul(
                m0, hidb[:, k, 0:TBB], cw_sb[:, k, 0:1])
            m1 = cvp.tile([128, TBB], bf16, name="cv")
            nc.vector.tensor_scalar(
                out=m1, in0=hidb[:, k, 1:1 + TBB],
                scalar1=cw_sb[:, k, 1:2], scalar2=cb_sb[:, k:k + 1],
                op0=OP.mult, op1=OP.add)
            t0_ = cvp.tile([128, TBB], bf16, name="cv")
            nc.vector.tensor_add(out=t0_, in0=m0, in1=m1)
            m2 = cvp.tile([128, TBB], bf16, name="cv")
            nc.vector.tensor_scalar_mul(
                m2, hidb[:, k, 2:2 + TBB], cw_sb[:, k, 2:3])
            m3 = cvp.tile([128, TBB], bf16, name="cv")
            nc.vector.tensor_scalar_mul(
                m3, hidb[:, k, 3:3 + TBB], cw_sb[:, k, 3:4])
            t1_ = cvp.tile([128, TBB], bf16, name="cv")
            nc.vector.tensor_add(out=t1_, in0=m2, in1=m3)
            t2_ = cvp.tile([128, TBB], bf16, name="cv")
            nc.vector.tensor_add(out=t2_, in0=t0_, in1=t1_)
            h2out = hid2[:, k, :]
            if OUT_FP8:
                h2out = h2out.bitcast(f8)
            nc.vector.tensor_mul(out=h2out, in0=t2_, in1=sigc)

        def emit_C_s(tb, s):
            t0 = tb * TBB
            pr = [psC.tile([128, 512], f32, name="pc") for _ in range(2)]
            if OUT_FP8:
                for j0 in range(KC // 2):
                    lh = hid2[:, 2 * j0:2 * j0 + 2,
                              s * 128:(s + 1) * 128].bitcast(f8)
                    for dh in range(2):
                        nc.tensor.matmul(
                            pr[dh], lh,
                            wout_sb[:, j0, :,
                                    dh * 512:(dh + 1) * 512].bitcast(f8),
                            start=(j0 == 0), stop=(j0 == KC // 2 - 1),
                            perf_mode=DR)
            else:
                for j in range(KC):
                    lh = hid2[:, j, s * 128:(s + 1) * 128]
                    for dh in range(2):
                        nc.tensor.matmul(
                            pr[dh], lh,
                            wout_sb[:, j, dh * 512:(dh + 1) * 512],
                            start=(j == 0), stop=(j == KC - 1))
            xr = xsp.tile([128, D], f32, name="xs")
            nc.sync.dma_start(
                out=xr, in_=xpb_d[t0 + s * 128:t0 + (s + 1) * 128, :])
            for dh in range(2):
                ot = otp.tile([128, 512], f32, name="ot")
                nc.vector.scalar_tensor_tensor(
                    out=ot, in0=pr[dh],
                    scalar=(1.0 / S_OUT) if OUT_FP8 else 1.0,
                    in1=xr[:, dh * 512:(dh + 1) * 512],
                    op0=OP.mult, op1=OP.add)
                nc.sync.dma_start(
                    out=out_d[t0 + s * 128:t0 + (s + 1) * 128,
                              dh * 512:(dh + 1) * 512],
                    in_=ot)

        sums_t, bcast_t, xT_t, x8_t = {}, {}, {}, {}

        # prologue: full phase A for tb=0
        sums_t[0] = sumsp.tile([128, 2, NSUB], f32, name="sums")
        for s in range(NSUB):
            emit_stats_s(0, s)
        emit_rows(0)
        x8_t[0] = x8p.tile([128, DC, TBB], u8, name="x8")
        for j in range(DC):
            emit_xt_dma(0, j)
            emit_norm_j(0, j)

        for tb in range(NTB):
            nxt = tb + 1 if tb + 1 < NTB else None
            if nxt is not None:
                sums_t[nxt] = sumsp.tile([128, 2, NSUB], f32, name="sums")
            for k in range(KC):
                emit_B_k(tb, k)
                if nxt is not None:
                    if k < NSUB:
                        emit_stats_s(nxt, k)
                    elif k == NSUB:
                        emit_rows(nxt)
                        x8_t[nxt] = x8p.tile([128, DC, TBB], u8, name="x8")
                    elif k >= 9:
                        if k == 9:
                            emit_xt_dma(nxt, 0)
                        emit_xt_dma(nxt, k - 9 + 1)
            if nxt is not None:
                emit_xt_dma(nxt, DC - 1)
            for s in range(NSUB):
                emit_C_s(tb, s)
                if nxt is not None and s < 4:
                    emit_norm_j(nxt, 2 * s)
                    emit_norm_j(nxt, 2 * s + 1)
            x8_t.pop(tb)
            # conv halo: copy last 3 hidden tokens to block front
            nc.vector.tensor_copy(out=hidb[:, :, 0:3],
                                  in_=hidb[:, :, TBB:TBB + 3])
    nc.compile()
    return nc


def _q8(a, scale):
    import ml_dtypes

    q = np.clip(a * scale, -240.0, 240.0).astype(ml_dtypes.float8_e4m3)
    return q.view(np.uint8)


def _prep_inputs(x, ln_g, ln_b, w_in, b_in, conv_w, conv_b, w_out, b_out):
    import ml_dtypes

    x = np.asarray(x, np.float32)
    ln_g = np.asarray(ln_g, np.float32)
    ln_b = np.asarray(ln_b, np.float32)
    w_in = np.asarray(w_in, np.float32)
    b_in = np.asarray(b_in, np.float32)
    conv_w = np.asarray(conv_w, np.float32)
    conv_b = np.asarray(conv_b, np.float32)
    w_out = np.asarray(w_out, np.float32)
    b_out = np.asarray(b_out, np.float32)

    wg = w_in * ln_g[None, :]                       # [3H, D]
    dv = b_in + w_in @ ln_b                         # [3H]
    # w_in8[p, k, t3, j0, i, m] = q8(wg[t3*2048+k*128+m, (2j0+i)*128+p]*S_IN)
    wblk = wg.reshape(3, KC, 128, DC // 2, 2, 128)  # [t3, k, m, j0, i, p]
    w_in8 = _q8(np.ascontiguousarray(wblk.transpose(5, 1, 0, 3, 4, 2)), S_IN)
    d_proc = np.ascontiguousarray(dv.reshape(3, KC, 128).transpose(2, 1, 0))
    d_proc[:, :, 2] *= S_IN
    cw_proc = np.ascontiguousarray(
        conv_w.reshape(KC, 128, KCONV).transpose(1, 0, 2)) / S_IN
    cb_proc = np.ascontiguousarray(conv_b.reshape(KC, 128).T)
    if OUT_FP8:
        # w_out8[p, j0, i, d] = q8(w_out[d, (2j0+i)*128+p] * S_OUT)
        wo = w_out.T.reshape(KC // 2, 2, 128, D)    # [j0, i, p, d]
        w_out_proc = _q8(np.ascontiguousarray(wo.transpose(2, 0, 1, 3)), S_OUT)
        wout_key = "w_out8"
    else:
        wo = w_out.T.reshape(KC, 128, D).transpose(1, 0, 2)
        w_out_proc = np.ascontiguousarray(wo).astype(ml_dtypes.bfloat16)
        wout_key = "w_outb"
    iden = np.eye(128, dtype=np.float32)

    shared = {
        "w_in8": w_in8, "d_proc": d_proc, "cw_proc": cw_proc,
        "cb_proc": cb_proc, wout_key: w_out_proc, "iden": iden,
    }
    maps = []
    for b in range(B):
        xb = np.ascontiguousarray(x[b])
        # xt_proc[tb, j, p, t] = x[b, TBB*tb + t, 128*j + p]
        xt_proc = np.ascontiguousarray(
            xb.T.reshape(DC, 128, NTB, TBB).transpose(2, 0, 1, 3)).astype(
                ml_dtypes.bfloat16)
        xpb = xb + b_out[None, :]
        maps.append(dict(shared, x=xb, xpb=xpb, xt_proc=xt_proc))
    return maps


def run(inputs, trace=False):
    _install_ntff_hook()
    import concourse.bass_utils as bu

    bu.upload_artifacts = lambda d: "local://" + d
    if "nc" not in _CACHE:
        _CACHE["nc"] = _build_nc()
    nc = _CACHE["nc"]
    in_maps = _prep_inputs(**inputs)
    res = bu.run_bass_kernel_spmd(nc, in_maps, list(range(N_CORES)),
                                  trace=trace)
    out = np.stack([res.results[b]["out"] for b in range(B)])
    return out.astype(np.float32), res


def kernel(**inputs):
    out, _ = run(inputs, trace=False)
    return out
